# revision 1
# baseline (speedup 1.0000x reference)
"""CfC cell (dense MLP) Trainium2 Bass kernel.

Reference math (fp32):
    x  = concat([input, hx], axis=1)                  # [B, 768]
    h  = 1.7159 * tanh(0.666 * (x @ Wb.T + bb))       # [B, 1024]
    ff1 = tanh(h @ W1.T + b1)                         # [B, 512]
    ff2 = tanh(h @ W2.T + b2)
    t_a = h @ Wa.T + ba
    t_b = h @ Wt.T + bt
    t   = sigmoid(t_a * ts + t_b)
    out = ff1 * (1 - t) + t * ff2

Strategy: data-parallel over batch across 8 NeuronCores (2048 rows each).
Host-side prep gives the device friendly layouts (fp16 matmul operands,
fp32 accumulation and elementwise):
  - xT        [768, 2048]   (x transposed -> contraction dim on partitions)
  - WbT       [768, 1024]   (Wb.T; stationary lhsT tiles for layer 1)
  - WH        [4, 1024, 512] (1.7159 * Wk.T; moving rhs for layer 2)
  - BBP       [128, 8]      (0.666*bb, per unit-tile columns; ACT bias)
  - BH        [4, 128, 512] (head biases broadcast across partitions)
  - TSP       [128, 16]     (ts, column mi = batch subtile mi)
Layer 1 produces hT [units, batch] tiles directly (no on-chip transposes);
layer 2 uses hT slices as the stationary operand producing [batch, hid]
output tiles, so ts becomes a per-partition scalar and the result DMAs out
with no transpose. Layer-1 runs one chunk ahead of layer-2 so the PE never
waits on the head-weight DMAs during startup.
"""

import os
import sys

import numpy as np

if "/opt/trn_rl_repo" not in sys.path:
    sys.path.insert(0, "/opt/trn_rl_repo")

B, IN, HID, UNITS = 16384, 256, 512, 1024
CAT = IN + HID  # 768
N_CORES = 8
BS = B // N_CORES  # 2048 per core
P = 128
NK1 = CAT // P    # 6 contraction tiles, layer 1
NU = UNITS // P   # 8 unit tiles
NH = 4            # heads

_cache = {}


def build_nc(bs=BS, chunk=512):
    """Build the single-core Bass program (same program runs SPMD on 8 cores)."""
    from concourse import bacc, tile, mybir

    AF = mybir.ActivationFunctionType
    ALU = mybir.AluOpType
    F32 = mybir.dt.float32
    F16 = mybir.dt.float16

    nchunk = bs // chunk
    nm = chunk // P  # batch subtiles per chunk

    nc = bacc.Bacc("TRN2", target_bir_lowering=False, debug=False,
                   num_devices=N_CORES)

    xt_d = nc.dram_tensor("xt", [CAT, bs], F16, kind="ExternalInput").ap()
    wbt_d = nc.dram_tensor("wbt", [CAT, UNITS], F16, kind="ExternalInput").ap()
    wh_d = nc.dram_tensor("wh", [NH, UNITS, HID], F16, kind="ExternalInput").ap()
    bbp_d = nc.dram_tensor("bbp", [P, NU], F32, kind="ExternalInput").ap()
    bh_d = nc.dram_tensor("bh", [NH, P, HID], F32, kind="ExternalInput").ap()
    tsp_d = nc.dram_tensor("tsp", [P, bs // P], F32, kind="ExternalInput").ap()
    out_d = nc.dram_tensor("out", [bs, HID], F32, kind="ExternalOutput").ap()

    with tile.TileContext(nc) as tc:
        with (
            tc.tile_pool(name="const", bufs=1) as const,
            tc.tile_pool(name="xp", bufs=4) as xp,
            tc.tile_pool(name="hp", bufs=4) as hp,
            tc.tile_pool(name="tp", bufs=2) as tp,
            tc.tile_pool(name="op", bufs=3) as op,
            tc.tile_pool(name="psp", bufs=8, space="PSUM") as psp,
        ):
            # --- PE warmup: keep HAM busy while startup DMAs stream ------
            warm = const.tile([P, 512], F16, tag="warm")
            nc.gpsimd.memset(warm[:], 0.0)
            for _ in range(8):
                wps = psp.tile([P, 512], F32, tag="ps")
                nc.tensor.matmul(wps[:], warm[:, 0:P], warm[:],
                                 start=True, stop=True)

            def load_x(bc):
                xts = []
                for c in range(NK1):
                    t = xp.tile([P, chunk], F16, tag=f"x{c}")
                    nc.sync.dma_start(
                        t[:], xt_d[c * P:(c + 1) * P, bc * chunk:(bc + 1) * chunk])
                    xts.append(t)
                return xts

            # first-chunk x tiles interleaved with the first weight half so
            # the c=0 accumulation group is runnable almost immediately
            HALF = UNITS // 2
            wb_sb = [[None, None] for _ in range(NK1)]
            xts0 = []
            for c in range(NK1):
                t = xp.tile([P, chunk], F16, tag=f"x{c}")
                nc.sync.dma_start(t[:], xt_d[c * P:(c + 1) * P, 0:chunk])
                xts0.append(t)
                t = const.tile([P, HALF], F16, tag=f"wbh{c}_0")
                nc.sync.dma_start(t[:], wbt_d[c * P:(c + 1) * P, 0:HALF])
                wb_sb[c][0] = t

            # small constants early (bb gates every layer-1 activation)
            bb_sb = const.tile([P, NU], F32, tag="bb")
            nc.sync.dma_start(bb_sb[:], bbp_d[:])
            ts_sb = const.tile([P, bs // P], F32, tag="ts")
            nc.sync.dma_start(ts_sb[:], tsp_d[:])

            for c in range(NK1):
                t = const.tile([P, HALF], F16, tag=f"wbh{c}_1")
                nc.sync.dma_start(
                    t[:], wbt_d[c * P:(c + 1) * P, HALF:UNITS])
                wb_sb[c][1] = t

            # all remaining x chunks next: layer-1 for every chunk runs
            # before any layer-2, so the head weights are needed only ~50us in
            xts_all = [xts0] + [load_x(bc) for bc in range(1, nchunk)]

            bh_sb = [None] * NH
            for k in range(NH):
                t = const.tile([P, HID], F32, tag=f"bh{k}", name=f"bh_{k}")
                nc.sync.dma_start(t[:], bh_d[k])
                bh_sb[k] = t

            wh_sb = [None] * NH

            def load_wh(k, eng):
                row = []
                for u in range(NU):
                    t = const.tile([P, HID], F16, tag=f"wh{k}_{u}")
                    eng.dma_start(t[:], wh_d[k, u * P:(u + 1) * P, :])
                    row.append(t)
                wh_sb[k] = row

            load_wh(2, nc.sync)
            load_wh(3, nc.sync)
            load_wh(0, nc.sync)
            load_wh(1, nc.sync)

            def layer1(xts):
                """hT[u] = tanh(0.666*(WbT.T @ xT) + 0.666*bb), fp16 out.

                c-outer accumulation in two u-half-groups: the first matmul
                only needs xts[0] + wb half, so PE starts as soon as the
                first ~0.26 MB of DMA lands.
                """
                hts = []
                for h in range(2):
                    pss = [psp.tile([P, chunk], F32, tag="ps", name=f"psl1_{j}")
                           for j in range(NU // 2)]
                    for c in range(NK1):
                        for j in range(NU // 2):
                            nc.tensor.matmul(
                                pss[j][:],
                                wb_sb[c][h][:, j * P:(j + 1) * P],
                                xts[c][:],
                                start=(c == 0), stop=(c == NK1 - 1))
                    for j in range(NU // 2):
                        u = h * (NU // 2) + j
                        ht = hp.tile([P, chunk], F16, tag=f"h{u}")
                        nc.scalar.activation(ht[:], pss[j][:], AF.Tanh,
                                             bias=bb_sb[:, u:u + 1], scale=0.666)
                        hts.append(ht)
                return hts

            def layer2(hts, bc):
                for m in range(nm):
                    mi = bc * nm + m
                    last = (bc == nchunk - 1) and (m == nm - 1)

                    def mm_head(k):
                        ps = psp.tile([P, HID], F32, tag="ps")
                        for u in range(NU):
                            nc.tensor.matmul(
                                ps[:],
                                hts[u][:, m * P:(m + 1) * P],
                                wh_sb[k][u][:],
                                start=(u == 0), stop=(u == NU - 1))
                        return ps

                    # t-path heads first so the sigmoid chain overlaps the
                    # ff1/ff2 matmuls
                    pa = mm_head(2)
                    pb = mm_head(3)
                    ua = tp.tile([P, HID], F32, tag="ua")
                    nc.vector.tensor_add(ua[:], pa[:], bh_sb[2][:])
                    ub = tp.tile([P, HID], F32, tag="ub")
                    nc.vector.tensor_add(ub[:], pb[:], bh_sb[3][:])
                    w = tp.tile([P, HID], F32, tag="w")
                    nc.vector.scalar_tensor_tensor(
                        w[:], ua[:], ts_sb[:, mi:mi + 1], ub[:],
                        op0=ALU.mult, op1=ALU.add)
                    tt = tp.tile([P, HID], F32, tag="tt")
                    nc.scalar.activation(tt[:], w[:], AF.Sigmoid)

                    p1 = mm_head(0)
                    u1 = tp.tile([P, HID], F32, tag="u1")
                    nc.vector.tensor_add(u1[:], p1[:], bh_sb[0][:])
                    f1 = tp.tile([P, HID], F32, tag="f1")
                    nc.scalar.activation(f1[:], u1[:], AF.Tanh)

                    p2 = mm_head(1)
                    o = op.tile([P, HID], F32, tag="o")
                    f2 = tp.tile([P, HID], F32, tag="f2")
                    # split the trailing chain into column halves on the very
                    # last tile so ACT/DVE pipeline instead of serializing
                    cols = ((slice(0, HID // 2), slice(HID // 2, HID))
                            if last else (slice(0, HID),))
                    for cs in cols:
                        u2 = tp.tile([P, HID], F32, tag="u2")
                        nc.vector.tensor_add(u2[:, cs], p2[:, cs], bh_sb[1][:, cs])
                        nc.scalar.activation(f2[:, cs], u2[:, cs], AF.Tanh)
                        # o = f1 + tt*(f2 - f1)
                        nc.vector.tensor_sub(o[:, cs], f2[:, cs], f1[:, cs])
                        nc.vector.tensor_mul(o[:, cs], o[:, cs], tt[:, cs])
                        nc.vector.tensor_add(o[:, cs], o[:, cs], f1[:, cs])
                        nc.sync.dma_start(out_d[mi * P:(mi + 1) * P, cs], o[:, cs])

            # --- all layer-1 chunks first, then all layer-2 --------------
            hts_all = [layer1(x) for x in xts_all]
            for bc in range(nchunk):
                layer2(hts_all[bc], bc)

    nc.compile()
    return nc


def _prep_inputs(input, hx, ts, Wb, bb, W1, b1, W2, b2, Wa, ba, Wt, bt, bs=BS,
                 n_cores=N_CORES):
    f = np.float32
    h = np.float16
    x = np.concatenate([np.asarray(input, f), np.asarray(hx, f)], axis=1)
    WbT = np.ascontiguousarray(np.asarray(Wb, f).T.astype(h))   # [768, 1024]
    WH = np.stack([np.ascontiguousarray((1.7159 * np.asarray(W, f)).T.astype(h))
                   for W in (W1, W2, Wa, Wt)])                  # [4, 1024, 512]
    BBP = np.ascontiguousarray(
        (0.666 * np.asarray(bb, f)).reshape(NU, P).T)           # [128, 8]
    BH = np.stack([np.ascontiguousarray(np.broadcast_to(np.asarray(b, f), (P, HID)))
                   for b in (b1, b2, ba, bt)])                  # [4, 128, 512]
    ts = np.asarray(ts, f).reshape(-1)
    xh = x.astype(h)

    in_maps = []
    for c in range(n_cores):
        lo, hi = c * bs, (c + 1) * bs
        in_maps.append({
            "xt": np.ascontiguousarray(xh[lo:hi].T),            # [768, bs] fp16
            "wbt": WbT,
            "wh": WH,
            "bbp": BBP,
            "bh": BH,
            "tsp": np.ascontiguousarray(ts[lo:hi].reshape(bs // P, P).T),
        })
    return in_maps


def kernel(input, hx, ts, Wb, bb, W1, b1, W2, b2, Wa, ba, Wt, bt):
    from concourse.bass_utils import run_bass_kernel_spmd

    if "nc" not in _cache:
        _cache["nc"] = build_nc()
    nc = _cache["nc"]

    in_maps = _prep_inputs(input, hx, ts, Wb, bb, W1, b1, W2, b2, Wa, ba, Wt, bt)
    trace = bool(int(os.environ.get("KERNEL_PROFILE", "0")))
    res = run_bass_kernel_spmd(nc, in_maps, list(range(N_CORES)), trace=trace)
    _cache["last_exec_time_ns"] = res.exec_time_ns
    _cache["last_results"] = res

    out = np.concatenate([res.results[c]["out"] for c in range(N_CORES)], axis=0)
    return out.astype(np.float32)



# revision 5
# speedup vs baseline: 1.1844x; 1.1844x over previous
"""CfC cell (dense MLP) Trainium2 Bass kernel.

Reference math (fp32):
    x  = concat([input, hx], axis=1)                  # [B, 768]
    h  = 1.7159 * tanh(0.666 * (x @ Wb.T + bb))       # [B, 1024]
    ff1 = tanh(h @ W1.T + b1)                         # [B, 512]
    ff2 = tanh(h @ W2.T + b2)
    t_a = h @ Wa.T + ba
    t_b = h @ Wt.T + bt
    t   = sigmoid(t_a * ts + t_b)
    out = ff1 * (1 - t) + t * ff2

Strategy: data-parallel over batch across 8 NeuronCores (2048 rows each).
Device layouts (contraction dim on partitions, fp16 matmul operands):
  - xT   [768, 2048]        x transposed
  - WbT  [768, 1024]        layer-1 stationary
  - WHF  [2, 1024, 512]     1.7159*W{1,2}.T, ff-head moving operands (fp16)
  - WH8  [2, 4, 128, 2, 512] K8*1.7159*W{a,t}.T as fp8e4m3 DoubleRow pairs
  - BBP  [128, 8]           0.666*bb (ACT bias for layer-1 tanh)
  - BHF/BH8 [2, 128, 512]   head biases broadcast across partitions (BH8 scaled K8)
  - TSP  [128, 16]          ts, column mi = batch subtile mi

Layer 1 emits hT [units, batch] twice from PSUM: fp16 tiles (ff heads) and
fp8 pair tiles (t heads).  The t-gate heads run as fp8 DoubleRow matmuls
(~1.44x PE rate); sigmoid's <=0.25 slope keeps the fp8 noise well inside the
accuracy budget.  All head biases are preloaded into PSUM by the (otherwise
idle) Pool engine so neither DVE nor ACT spends time on bias adds; the
accumulation groups then run start=False.  The final interpolation runs in
fp16 on DVE (2x rate).
"""

import os
import sys

import numpy as np

if "/opt/trn_rl_repo" not in sys.path:
    sys.path.insert(0, "/opt/trn_rl_repo")

B, IN, HID, UNITS = 16384, 256, 512, 1024
CAT = IN + HID  # 768
N_CORES = 8
BS = B // N_CORES  # 2048 per core
P = 128
NK1 = CAT // P    # 6 contraction tiles, layer 1
NU = UNITS // P   # 8 unit tiles
NJ = NU // 2      # 4 unit pair tiles (DoubleRow)
K8 = 1024.0       # fp8 weight pre-scale (sigmoid applies 1/K8)

_cache = {}


def build_nc(bs=BS, chunk=512):
    """Build the single-core Bass program (same program runs SPMD on 8 cores)."""
    from concourse import bacc, tile, mybir

    AF = mybir.ActivationFunctionType
    ALU = mybir.AluOpType
    PM = mybir.MatmulPerfMode
    F32 = mybir.dt.float32
    F16 = mybir.dt.float16
    F8 = mybir.dt.float8e4

    nchunk = bs // chunk
    nm = chunk // P  # batch subtiles per chunk
    NM = bs // P     # total batch subtiles

    nc = bacc.Bacc("TRN2", target_bir_lowering=False, debug=False,
                   num_devices=N_CORES)

    xt_d = nc.dram_tensor("xt", [CAT, bs], F16, kind="ExternalInput").ap()
    wbt_d = nc.dram_tensor("wbt", [CAT, UNITS], F16, kind="ExternalInput").ap()
    whf_d = nc.dram_tensor("whf", [2, UNITS, HID], F16, kind="ExternalInput").ap()
    wh8_d = nc.dram_tensor("wh8", [2, NJ, P, 2, HID], F8, kind="ExternalInput").ap()
    bbp_d = nc.dram_tensor("bbp", [P, NU], F32, kind="ExternalInput").ap()
    bhf_d = nc.dram_tensor("bhf", [2, P, HID], F32, kind="ExternalInput").ap()
    bh8_d = nc.dram_tensor("bh8", [2, P, HID], F32, kind="ExternalInput").ap()
    tsp_d = nc.dram_tensor("tsp", [P, NM], F32, kind="ExternalInput").ap()
    out_d = nc.dram_tensor("out", [bs, HID], F32, kind="ExternalOutput").ap()

    with tile.TileContext(nc) as tc:
        with (
            tc.tile_pool(name="const", bufs=1) as const,
            tc.tile_pool(name="xp", bufs=4) as xp,
            tc.tile_pool(name="hp", bufs=1) as hp,
            tc.tile_pool(name="tp", bufs=2) as tp,
            tc.tile_pool(name="op", bufs=3) as op,
            tc.tile_pool(name="psp", bufs=8, space="PSUM") as psp,
        ):
            # --- PE warmup: ramp the p-state while startup DMAs stream ----
            warm = const.tile([P, 512], F16, tag="warm")
            nc.vector.memset(warm[:], 0.0)
            for _ in range(4):
                wps = psp.tile([P, 512], F32, tag="ps")
                nc.tensor.matmul(wps[:], warm[:, 0:P], warm[:],
                                 start=True, stop=True)

            def load_x(bc):
                xts = []
                for c in range(NK1):
                    t = xp.tile([P, chunk], F16, tag=f"x{c}")
                    nc.sync.dma_start(
                        t[:], xt_d[c * P:(c + 1) * P, bc * chunk:(bc + 1) * chunk])
                    xts.append(t)
                return xts

            # first-chunk x tiles interleaved with the first weight half so
            # the c=0 accumulation group is runnable almost immediately
            HALF = UNITS // 2
            wb_sb = [[None, None] for _ in range(NK1)]
            xts0 = []
            for c in range(NK1):
                t = xp.tile([P, chunk], F16, tag=f"x{c}")
                nc.sync.dma_start(t[:], xt_d[c * P:(c + 1) * P, 0:chunk])
                xts0.append(t)
                t = const.tile([P, HALF], F16, tag=f"wbh{c}_0")
                nc.sync.dma_start(t[:], wbt_d[c * P:(c + 1) * P, 0:HALF])
                wb_sb[c][0] = t

            # small constants early (bb gates every layer-1 activation)
            bb_sb = const.tile([P, NU], F32, tag="bb")
            nc.sync.dma_start(bb_sb[:], bbp_d[:])

            for c in range(NK1):
                t = const.tile([P, HALF], F16, tag=f"wbh{c}_1")
                nc.sync.dma_start(
                    t[:], wbt_d[c * P:(c + 1) * P, HALF:UNITS])
                wb_sb[c][1] = t

            # all remaining x chunks next: layer-1 for every chunk runs
            # before any layer-2, so the head weights are needed only ~45us in
            xts_all = [xts0] + [load_x(bc) for bc in range(1, nchunk)]

            # t-head fp8 weights (needed first in each layer-2 m-tile)
            wh8_sb = [[None] * NJ for _ in range(2)]
            for k in range(2):
                for j in range(NJ):
                    t = const.tile([P, 2, HID], F8, tag=f"wh8_{k}_{j}")
                    nc.sync.dma_start(t[:], wh8_d[k, j])
                    wh8_sb[k][j] = t

            # ff-head fp16 weights
            whf_sb = [[None] * NU for _ in range(2)]
            for k in range(2):
                for u in range(NU):
                    t = const.tile([P, HID], F16, tag=f"whf_{k}_{u}")
                    nc.sync.dma_start(t[:], whf_d[k, u * P:(u + 1) * P, :])
                    whf_sb[k][u] = t

            # biases + ts (gate the first layer-2 psum preloads / stt)
            bh8_sb = [None, None]
            bhf_sb = [None, None]
            for k in range(2):
                t = const.tile([P, HID], F32, tag=f"bh8_{k}", name=f"bh8_{k}")
                nc.sync.dma_start(t[:], bh8_d[k])
                bh8_sb[k] = t
            for k in range(2):
                t = const.tile([P, HID], F32, tag=f"bhf_{k}", name=f"bhf_{k}")
                nc.sync.dma_start(t[:], bhf_d[k])
                bhf_sb[k] = t
            ts_sb = const.tile([P, NM], F32, tag="ts")
            nc.sync.dma_start(ts_sb[:], tsp_d[:])

            # persistent h storage: fp16 per u-tile, fp8 pairs per j-tile
            h16 = [[None] * NU for _ in range(nchunk)]
            h8 = [[None] * NJ for _ in range(nchunk)]
            for bc in range(nchunk):
                for u in range(NU):
                    h16[bc][u] = hp.tile([P, chunk], F16, tag=f"h16_{bc}_{u}",
                                         name=f"h16_{bc}_{u}")
                for j in range(NJ):
                    h8[bc][j] = hp.tile([P, 2, chunk], F8, tag=f"h8_{bc}_{j}",
                                        name=f"h8_{bc}_{j}")

            def layer1(bc):
                """hT[u] = tanh(0.666*(WbT.T @ xT) + 0.666*bb), fp16 + fp8 out.

                c-outer accumulation in two u-half-groups: the first matmul
                only needs xts[0] + wb half, so PE starts as soon as the
                first ~0.26 MB of DMA lands.
                """
                xts = xts_all[bc]
                for half in range(2):
                    pss = [psp.tile([P, chunk], F32, tag="ps", name=f"psl1_{j}")
                           for j in range(NU // 2)]
                    for c in range(NK1):
                        for j in range(NU // 2):
                            nc.tensor.matmul(
                                pss[j][:],
                                wb_sb[c][half][:, j * P:(j + 1) * P],
                                xts[c][:],
                                start=(c == 0), stop=(c == NK1 - 1))
                    for j in range(NU // 2):
                        u = half * (NU // 2) + j
                        nc.scalar.activation(h16[bc][u][:], pss[j][:], AF.Tanh,
                                             bias=bb_sb[:, u:u + 1], scale=0.666)
                        nc.scalar.activation(h8[bc][u // 2][:, u % 2, :],
                                             pss[j][:], AF.Tanh,
                                             bias=bb_sb[:, u:u + 1], scale=0.666)

            def layer2(bc):
                hts = h16[bc]
                h8s = h8[bc]
                for m in range(nm):
                    mi = bc * nm + m
                    last = (bc == nchunk - 1) and (m == nm - 1)
                    mc = slice(m * P, (m + 1) * P)

                    # t-path heads (fp8 DoubleRow), biases preloaded by Pool
                    def mm_t(k):
                        ps = psp.tile([P, HID], F32, tag="ps")
                        nc.vector.tensor_copy(ps[:], bh8_sb[k][:])
                        for j in range(NJ):
                            nc.tensor.matmul(
                                ps[:],
                                h8s[j][:, :, mc],
                                wh8_sb[k][j][:],
                                start=False, stop=(j == NJ - 1),
                                perf_mode=PM.DoubleRow,
                                skip_group_check=True)
                        return ps

                    def mm_ff(k):
                        ps = psp.tile([P, HID], F32, tag="ps")
                        nc.vector.tensor_copy(ps[:], bhf_sb[k][:])
                        for u in range(NU):
                            nc.tensor.matmul(
                                ps[:],
                                hts[u][:, mc],
                                whf_sb[k][u][:],
                                start=False, stop=(u == NU - 1),
                                skip_group_check=True)
                        return ps

                    # t-path first so the sigmoid chain overlaps the ff matmuls.
                    # DVE may read only one PSUM operand per op: stage pb in
                    # SBUF via ACT (overlaps pa's matmuls).
                    pb = mm_t(1)
                    ub = tp.tile([P, HID], F32, tag="ub")
                    nc.scalar.copy(ub[:], pb[:])
                    pa = mm_t(0)
                    w = tp.tile([P, HID], F32, tag="w")
                    nc.vector.scalar_tensor_tensor(
                        w[:], pa[:], ts_sb[:, mi:mi + 1], ub[:],
                        op0=ALU.mult, op1=ALU.add)
                    tt = tp.tile([P, HID], F16, tag="tt")
                    nc.scalar.activation(tt[:], w[:], AF.Sigmoid, scale=1.0 / K8)

                    p1 = mm_ff(0)
                    f1 = tp.tile([P, HID], F16, tag="f1")
                    nc.scalar.activation(f1[:], p1[:], AF.Tanh)

                    p2 = mm_ff(1)
                    f2 = tp.tile([P, HID], F16, tag="f2")
                    o = op.tile([P, HID], F32, tag="o")
                    # split the trailing chain into column quarters on the very
                    # last tile so ACT/DVE/DMA pipeline instead of serializing
                    cols = (tuple(slice(q * (HID // 4), (q + 1) * (HID // 4))
                                  for q in range(4))
                            if last else (slice(0, HID),))
                    for cs in cols:
                        d = tp.tile([P, HID], F16, tag="d")
                        nc.scalar.activation(f2[:, cs], p2[:, cs], AF.Tanh)
                        # o = f1 + tt*(f2 - f1)
                        nc.vector.tensor_sub(d[:, cs], f2[:, cs], f1[:, cs])
                        nc.vector.tensor_mul(d[:, cs], d[:, cs], tt[:, cs])
                        nc.vector.tensor_add(o[:, cs], d[:, cs], f1[:, cs])
                        nc.sync.dma_start(out_d[mi * P:(mi + 1) * P, cs], o[:, cs])

            # --- all layer-1 chunks first, then all layer-2 --------------
            for bc in range(nchunk):
                layer1(bc)
            for bc in range(nchunk):
                layer2(bc)

    nc.compile()
    return nc


def _prep_inputs(input, hx, ts, Wb, bb, W1, b1, W2, b2, Wa, ba, Wt, bt, bs=BS,
                 n_cores=N_CORES):
    import ml_dtypes
    f = np.float32
    h = np.float16
    f8 = ml_dtypes.float8_e4m3
    x = np.concatenate([np.asarray(input, f), np.asarray(hx, f)], axis=1)
    WbT = np.ascontiguousarray(np.asarray(Wb, f).T.astype(h))   # [768, 1024]
    WHF = np.stack([np.ascontiguousarray((1.7159 * np.asarray(W, f)).T.astype(h))
                    for W in (W1, W2)])                         # [2, 1024, 512]
    # fp8 DoubleRow pair layout: unit u = j*256 + s*128 + p -> [j, p, s, hid]
    WH8 = np.stack([
        np.ascontiguousarray(
            (K8 * 1.7159 * np.asarray(W, f)).T
            .reshape(NJ, 2, P, HID).transpose(0, 2, 1, 3).astype(f8))
        for W in (Wa, Wt)])                                     # [2, 4, 128, 2, 512]
    BBP = np.ascontiguousarray(
        (0.666 * np.asarray(bb, f)).reshape(NU, P).T)           # [128, 8]
    BHF = np.stack([np.ascontiguousarray(np.broadcast_to(np.asarray(b, f), (P, HID)))
                    for b in (b1, b2)])                         # [2, 128, 512]
    BH8 = np.stack([np.ascontiguousarray(np.broadcast_to(
        (K8 * np.asarray(b, f)).astype(f), (P, HID)))
        for b in (ba, bt)])                                     # [2, 128, 512]
    ts = np.asarray(ts, f).reshape(-1)
    xh = x.astype(h)

    in_maps = []
    for c in range(n_cores):
        lo, hi = c * bs, (c + 1) * bs
        in_maps.append({
            "xt": np.ascontiguousarray(xh[lo:hi].T),            # [768, bs] fp16
            "wbt": WbT,
            "whf": WHF,
            "wh8": WH8,
            "bbp": BBP,
            "bhf": BHF,
            "bh8": BH8,
            "tsp": np.ascontiguousarray(ts[lo:hi].reshape(bs // P, P).T),
        })
    return in_maps


def kernel(input, hx, ts, Wb, bb, W1, b1, W2, b2, Wa, ba, Wt, bt):
    from concourse.bass_utils import run_bass_kernel_spmd

    if "nc" not in _cache:
        _cache["nc"] = build_nc()
    nc = _cache["nc"]

    in_maps = _prep_inputs(input, hx, ts, Wb, bb, W1, b1, W2, b2, Wa, ba, Wt, bt)
    trace = bool(int(os.environ.get("KERNEL_PROFILE", "0")))
    res = run_bass_kernel_spmd(nc, in_maps, list(range(N_CORES)), trace=trace)
    _cache["last_exec_time_ns"] = res.exec_time_ns
    _cache["last_results"] = res

    out = np.concatenate([res.results[c]["out"] for c in range(N_CORES)], axis=0)
    return out.astype(np.float32)


# revision 8
# speedup vs baseline: 1.1869x; 1.0021x over previous
"""CfC cell (dense MLP) Trainium2 Bass kernel.

Reference math (fp32):
    x  = concat([input, hx], axis=1)                  # [B, 768]
    h  = 1.7159 * tanh(0.666 * (x @ Wb.T + bb))       # [B, 1024]
    ff1 = tanh(h @ W1.T + b1)                         # [B, 512]
    ff2 = tanh(h @ W2.T + b2)
    t_a = h @ Wa.T + ba
    t_b = h @ Wt.T + bt
    t   = sigmoid(t_a * ts + t_b)
    out = ff1 * (1 - t) + t * ff2

Strategy: data-parallel over batch across 8 NeuronCores (2048 rows each).
Device layouts (contraction dim on partitions, fp16 matmul operands):
  - xT   [768, 2048]        x transposed
  - WbT  [768, 1024]        layer-1 stationary
  - WHF  [2, 1024, 512]     1.7159*W{1,2}.T, ff-head moving operands (fp16)
  - WH8  [2, 4, 128, 2, 512] K8*1.7159*W{a,t}.T as fp8e4m3 DoubleRow pairs
  - BBP  [128, 8]           0.666*bb (ACT bias for layer-1 tanh)
  - BHF/BH8 [2, 128, 512]   head biases broadcast across partitions (BH8 scaled K8)
  - TSP  [128, 16]          ts, column mi = batch subtile mi

Layer 1 emits hT [units, batch] twice from PSUM: fp16 tiles (ff heads) and
fp8 pair tiles (t heads).  The t-gate heads run as fp8 DoubleRow matmuls
(~1.44x PE rate); sigmoid's <=0.25 slope keeps the fp8 noise well inside the
accuracy budget.  All head biases are preloaded into PSUM by the (otherwise
idle) Pool engine so neither DVE nor ACT spends time on bias adds; the
accumulation groups then run start=False.  The final interpolation runs in
fp16 on DVE (2x rate).
"""

import os
import sys

import numpy as np

if "/opt/trn_rl_repo" not in sys.path:
    sys.path.insert(0, "/opt/trn_rl_repo")

B, IN, HID, UNITS = 16384, 256, 512, 1024
CAT = IN + HID  # 768
N_CORES = 8
BS = B // N_CORES  # 2048 per core
P = 128
NK1 = CAT // P    # 6 contraction tiles, layer 1
NU = UNITS // P   # 8 unit tiles
NJ = NU // 2      # 4 unit pair tiles (DoubleRow)
K8 = 1024.0       # fp8 weight pre-scale (sigmoid applies 1/K8)

_cache = {}


def build_nc(bs=BS, chunk=512):
    """Build the single-core Bass program (same program runs SPMD on 8 cores)."""
    from concourse import bacc, tile, mybir

    AF = mybir.ActivationFunctionType
    ALU = mybir.AluOpType
    PM = mybir.MatmulPerfMode
    F32 = mybir.dt.float32
    F16 = mybir.dt.float16
    F8 = mybir.dt.float8e4

    nchunk = bs // chunk
    nm = chunk // P  # batch subtiles per chunk
    NM = bs // P     # total batch subtiles

    nc = bacc.Bacc("TRN2", target_bir_lowering=False, debug=False,
                   num_devices=N_CORES)

    xt_d = nc.dram_tensor("xt", [CAT, bs], F16, kind="ExternalInput").ap()
    wbt_d = nc.dram_tensor("wbt", [CAT, UNITS], F16, kind="ExternalInput").ap()
    whf_d = nc.dram_tensor("whf", [2, UNITS, HID], F16, kind="ExternalInput").ap()
    wh8_d = nc.dram_tensor("wh8", [2, NJ, P, 2, HID], F8, kind="ExternalInput").ap()
    bbp_d = nc.dram_tensor("bbp", [P, NU], F32, kind="ExternalInput").ap()
    bhf_d = nc.dram_tensor("bhf", [2, P, HID], F32, kind="ExternalInput").ap()
    bh8_d = nc.dram_tensor("bh8", [2, P, HID], F32, kind="ExternalInput").ap()
    tsp_d = nc.dram_tensor("tsp", [P, NM], F32, kind="ExternalInput").ap()
    out_d = nc.dram_tensor("out", [bs, HID], F32, kind="ExternalOutput").ap()

    with tile.TileContext(nc) as tc:
        with (
            tc.tile_pool(name="const", bufs=1) as const,
            tc.tile_pool(name="xp", bufs=4) as xp,
            tc.tile_pool(name="hp", bufs=1) as hp,
            tc.tile_pool(name="tp", bufs=2) as tp,
            tc.tile_pool(name="op", bufs=3) as op,
            tc.tile_pool(name="psp", bufs=8, space="PSUM") as psp,
        ):
            # --- PE warmup: ramp the p-state while startup DMAs stream ----
            warm = const.tile([P, 512], F16, tag="warm")
            nc.vector.memset(warm[:], 0.0)
            for _ in range(3):
                wps = psp.tile([P, 512], F32, tag="ps")
                nc.tensor.matmul(wps[:], warm[:, 0:P], warm[:],
                                 start=True, stop=True)

            def load_x(bc):
                xts = []
                for c in range(NK1):
                    t = xp.tile([P, chunk], F16, tag=f"x{c}")
                    nc.sync.dma_start(
                        t[:], xt_d[c * P:(c + 1) * P, bc * chunk:(bc + 1) * chunk])
                    xts.append(t)
                return xts

            # first-chunk x tiles on the Sync DGE queue, layer-1 weights on
            # the Scalar DGE queue (idle until layer-1 ACT ~18us in): both
            # issue streams run in parallel so chunk 0 is fully resident
            # ~3.5us sooner than a single serialized queue
            HALF = UNITS // 2
            wb_sb = [[None, None] for _ in range(NK1)]
            xts0 = []
            for c in range(NK1):
                t = xp.tile([P, chunk], F16, tag=f"x{c}")
                nc.sync.dma_start(t[:], xt_d[c * P:(c + 1) * P, 0:chunk])
                xts0.append(t)
                t = const.tile([P, HALF], F16, tag=f"wbh{c}_0")
                nc.scalar.dma_start(t[:], wbt_d[c * P:(c + 1) * P, 0:HALF])
                wb_sb[c][0] = t

            # small constants early (bb gates every layer-1 activation)
            bb_sb = const.tile([P, NU], F32, tag="bb")
            nc.scalar.dma_start(bb_sb[:], bbp_d[:])

            for c in range(NK1):
                t = const.tile([P, HALF], F16, tag=f"wbh{c}_1")
                nc.scalar.dma_start(
                    t[:], wbt_d[c * P:(c + 1) * P, HALF:UNITS])
                wb_sb[c][1] = t

            # all remaining x chunks next: layer-1 for every chunk runs
            # before any layer-2, so the head weights are needed only ~45us in
            xts_all = [xts0] + [load_x(bc) for bc in range(1, nchunk)]

            # t-head fp8 weights (needed first in each layer-2 m-tile)
            wh8_sb = [[None] * NJ for _ in range(2)]
            for k in range(2):
                for j in range(NJ):
                    t = const.tile([P, 2, HID], F8, tag=f"wh8_{k}_{j}")
                    nc.sync.dma_start(t[:], wh8_d[k, j])
                    wh8_sb[k][j] = t

            # ff-head fp16 weights
            whf_sb = [[None] * NU for _ in range(2)]
            for k in range(2):
                for u in range(NU):
                    t = const.tile([P, HID], F16, tag=f"whf_{k}_{u}")
                    nc.sync.dma_start(t[:], whf_d[k, u * P:(u + 1) * P, :])
                    whf_sb[k][u] = t

            # biases + ts (gate the first layer-2 psum preloads / stt)
            bh8_sb = [None, None]
            bhf_sb = [None, None]
            for k in range(2):
                t = const.tile([P, HID], F32, tag=f"bh8_{k}", name=f"bh8_{k}")
                nc.sync.dma_start(t[:], bh8_d[k])
                bh8_sb[k] = t
            for k in range(2):
                t = const.tile([P, HID], F32, tag=f"bhf_{k}", name=f"bhf_{k}")
                nc.sync.dma_start(t[:], bhf_d[k])
                bhf_sb[k] = t
            ts_sb = const.tile([P, NM], F32, tag="ts")
            nc.sync.dma_start(ts_sb[:], tsp_d[:])

            # persistent h storage: fp16 per u-tile, fp8 pairs per j-tile
            h16 = [[None] * NU for _ in range(nchunk)]
            h8 = [[None] * NJ for _ in range(nchunk)]
            for bc in range(nchunk):
                for u in range(NU):
                    h16[bc][u] = hp.tile([P, chunk], F16, tag=f"h16_{bc}_{u}",
                                         name=f"h16_{bc}_{u}")
                for j in range(NJ):
                    h8[bc][j] = hp.tile([P, 2, chunk], F8, tag=f"h8_{bc}_{j}",
                                        name=f"h8_{bc}_{j}")

            def layer1(bc):
                """hT[u] = tanh(0.666*(WbT.T @ xT) + 0.666*bb), fp16 + fp8 out.

                c-outer accumulation in two u-half-groups: the first matmul
                only needs xts[0] + wb half, so PE starts as soon as the
                first ~0.26 MB of DMA lands.
                """
                xts = xts_all[bc]
                for half in range(2):
                    pss = [psp.tile([P, chunk], F32, tag="ps", name=f"psl1_{j}")
                           for j in range(NU // 2)]
                    for c in range(NK1):
                        for j in range(NU // 2):
                            nc.tensor.matmul(
                                pss[j][:],
                                wb_sb[c][half][:, j * P:(j + 1) * P],
                                xts[c][:],
                                start=(c == 0), stop=(c == NK1 - 1))
                    for j in range(NU // 2):
                        u = half * (NU // 2) + j
                        nc.scalar.activation(h16[bc][u][:], pss[j][:], AF.Tanh,
                                             bias=bb_sb[:, u:u + 1], scale=0.666)
                        nc.scalar.activation(h8[bc][u // 2][:, u % 2, :],
                                             pss[j][:], AF.Tanh,
                                             bias=bb_sb[:, u:u + 1], scale=0.666)

            def layer2(bc):
                hts = h16[bc]
                h8s = h8[bc]
                for m in range(nm):
                    mi = bc * nm + m
                    last = (bc == nchunk - 1) and (m == nm - 1)
                    mc = slice(m * P, (m + 1) * P)

                    # t-path heads (fp8 DoubleRow), biases preloaded by Pool
                    def mm_t(k):
                        ps = psp.tile([P, HID], F32, tag="ps")
                        nc.vector.tensor_copy(ps[:], bh8_sb[k][:])
                        for j in range(NJ):
                            nc.tensor.matmul(
                                ps[:],
                                h8s[j][:, :, mc],
                                wh8_sb[k][j][:],
                                start=False, stop=(j == NJ - 1),
                                perf_mode=PM.DoubleRow,
                                skip_group_check=True)
                        return ps

                    def mm_ff(k):
                        ps = psp.tile([P, HID], F32, tag="ps")
                        nc.vector.tensor_copy(ps[:], bhf_sb[k][:])
                        for u in range(NU):
                            nc.tensor.matmul(
                                ps[:],
                                hts[u][:, mc],
                                whf_sb[k][u][:],
                                start=False, stop=(u == NU - 1),
                                skip_group_check=True)
                        return ps

                    # t-path first so the sigmoid chain overlaps the ff matmuls.
                    # DVE may read only one PSUM operand per op: stage pb in
                    # SBUF via ACT (overlaps pa's matmuls).
                    pb = mm_t(1)
                    ub = tp.tile([P, HID], F32, tag="ub")
                    nc.scalar.copy(ub[:], pb[:])
                    pa = mm_t(0)
                    w = tp.tile([P, HID], F32, tag="w")
                    nc.vector.scalar_tensor_tensor(
                        w[:], pa[:], ts_sb[:, mi:mi + 1], ub[:],
                        op0=ALU.mult, op1=ALU.add)
                    tt = tp.tile([P, HID], F16, tag="tt")
                    nc.scalar.activation(tt[:], w[:], AF.Sigmoid, scale=1.0 / K8)

                    p1 = mm_ff(0)
                    f1 = tp.tile([P, HID], F16, tag="f1")
                    nc.scalar.activation(f1[:], p1[:], AF.Tanh)

                    p2 = mm_ff(1)
                    f2 = tp.tile([P, HID], F16, tag="f2")
                    o = op.tile([P, HID], F32, tag="o")
                    # split the trailing chain into column quarters on the very
                    # last tile so ACT/DVE/DMA pipeline instead of serializing
                    cols = (tuple(slice(q * (HID // 4), (q + 1) * (HID // 4))
                                  for q in range(4))
                            if last else (slice(0, HID),))
                    for qi, cs in enumerate(cols):
                        d = tp.tile([P, HID], F16, tag="d")
                        nc.scalar.activation(f2[:, cs], p2[:, cs], AF.Tanh)
                        # o = f1 + tt*(f2 - f1)
                        nc.vector.tensor_sub(d[:, cs], f2[:, cs], f1[:, cs])
                        nc.vector.tensor_mul(d[:, cs], d[:, cs], tt[:, cs])
                        nc.vector.tensor_add(o[:, cs], d[:, cs], f1[:, cs])
                        # alternate DGE queues so the tail quarters' DMA
                        # issues (~600ns each) overlap instead of serializing
                        eng = nc.scalar if (last and qi % 2) else nc.sync
                        eng.dma_start(out_d[mi * P:(mi + 1) * P, cs], o[:, cs])

            # --- all layer-1 chunks first, then all layer-2 --------------
            for bc in range(nchunk):
                layer1(bc)
            for bc in range(nchunk):
                layer2(bc)

    nc.compile()
    return nc


def _prep_inputs(input, hx, ts, Wb, bb, W1, b1, W2, b2, Wa, ba, Wt, bt, bs=BS,
                 n_cores=N_CORES):
    import ml_dtypes
    f = np.float32
    h = np.float16
    f8 = ml_dtypes.float8_e4m3
    x = np.concatenate([np.asarray(input, f), np.asarray(hx, f)], axis=1)
    WbT = np.ascontiguousarray(np.asarray(Wb, f).T.astype(h))   # [768, 1024]
    WHF = np.stack([np.ascontiguousarray((1.7159 * np.asarray(W, f)).T.astype(h))
                    for W in (W1, W2)])                         # [2, 1024, 512]
    # fp8 DoubleRow pair layout: unit u = j*256 + s*128 + p -> [j, p, s, hid]
    WH8 = np.stack([
        np.ascontiguousarray(
            (K8 * 1.7159 * np.asarray(W, f)).T
            .reshape(NJ, 2, P, HID).transpose(0, 2, 1, 3).astype(f8))
        for W in (Wa, Wt)])                                     # [2, 4, 128, 2, 512]
    BBP = np.ascontiguousarray(
        (0.666 * np.asarray(bb, f)).reshape(NU, P).T)           # [128, 8]
    BHF = np.stack([np.ascontiguousarray(np.broadcast_to(np.asarray(b, f), (P, HID)))
                    for b in (b1, b2)])                         # [2, 128, 512]
    BH8 = np.stack([np.ascontiguousarray(np.broadcast_to(
        (K8 * np.asarray(b, f)).astype(f), (P, HID)))
        for b in (ba, bt)])                                     # [2, 128, 512]
    ts = np.asarray(ts, f).reshape(-1)
    xh = x.astype(h)

    in_maps = []
    for c in range(n_cores):
        lo, hi = c * bs, (c + 1) * bs
        in_maps.append({
            "xt": np.ascontiguousarray(xh[lo:hi].T),            # [768, bs] fp16
            "wbt": WbT,
            "whf": WHF,
            "wh8": WH8,
            "bbp": BBP,
            "bhf": BHF,
            "bh8": BH8,
            "tsp": np.ascontiguousarray(ts[lo:hi].reshape(bs // P, P).T),
        })
    return in_maps


def kernel(input, hx, ts, Wb, bb, W1, b1, W2, b2, Wa, ba, Wt, bt):
    from concourse.bass_utils import run_bass_kernel_spmd

    if "nc" not in _cache:
        _cache["nc"] = build_nc()
    nc = _cache["nc"]

    in_maps = _prep_inputs(input, hx, ts, Wb, bb, W1, b1, W2, b2, Wa, ba, Wt, bt)
    trace = bool(int(os.environ.get("KERNEL_PROFILE", "0")))
    res = run_bass_kernel_spmd(nc, in_maps, list(range(N_CORES)), trace=trace)
    _cache["last_exec_time_ns"] = res.exec_time_ns
    _cache["last_results"] = res

    out = np.concatenate([res.results[c]["out"] for c in range(N_CORES)], axis=0)
    return out.astype(np.float32)


# revision 11
# speedup vs baseline: 1.1887x; 1.0015x over previous
"""CfC cell (dense MLP) Trainium2 Bass kernel.

Reference math (fp32):
    x  = concat([input, hx], axis=1)                  # [B, 768]
    h  = 1.7159 * tanh(0.666 * (x @ Wb.T + bb))       # [B, 1024]
    ff1 = tanh(h @ W1.T + b1)                         # [B, 512]
    ff2 = tanh(h @ W2.T + b2)
    t_a = h @ Wa.T + ba
    t_b = h @ Wt.T + bt
    t   = sigmoid(t_a * ts + t_b)
    out = ff1 * (1 - t) + t * ff2

Strategy: data-parallel over batch across 8 NeuronCores (2048 rows each).
Device layouts (contraction dim on partitions, fp16 matmul operands):
  - xT   [768, 2048]        x transposed
  - WbT  [768, 1024]        layer-1 stationary
  - WHF  [2, 1024, 512]     1.7159*W{1,2}.T, ff-head moving operands (fp16)
  - WH8  [2, 4, 128, 2, 512] K8*1.7159*W{a,t}.T as fp8e4m3 DoubleRow pairs
  - BBP  [128, 8]           0.666*bb (ACT bias for layer-1 tanh)
  - BHF/BH8 [2, 128, 512]   head biases broadcast across partitions (BH8 scaled K8)
  - TSP  [128, 16]          ts, column mi = batch subtile mi

Layer 1 emits hT [units, batch] twice from PSUM: fp16 tiles (ff heads) and
fp8 pair tiles (t heads).  The t-gate heads run as fp8 DoubleRow matmuls
(~1.44x PE rate); sigmoid's <=0.25 slope keeps the fp8 noise well inside the
accuracy budget.  All head biases are preloaded into PSUM by the (otherwise
idle) Pool engine so neither DVE nor ACT spends time on bias adds; the
accumulation groups then run start=False.  The final interpolation runs in
fp16 on DVE (2x rate).
"""

import os
import sys

import numpy as np

if "/opt/trn_rl_repo" not in sys.path:
    sys.path.insert(0, "/opt/trn_rl_repo")

B, IN, HID, UNITS = 16384, 256, 512, 1024
CAT = IN + HID  # 768
N_CORES = 8
BS = B // N_CORES  # 2048 per core
P = 128
NK1 = CAT // P    # 6 contraction tiles, layer 1
NU = UNITS // P   # 8 unit tiles
NJ = NU // 2      # 4 unit pair tiles (DoubleRow)
K8 = 1024.0       # fp8 weight pre-scale (sigmoid applies 1/K8)

_cache = {}


def build_nc(bs=BS, chunk=512):
    """Build the single-core Bass program (same program runs SPMD on 8 cores)."""
    from concourse import bacc, tile, mybir

    AF = mybir.ActivationFunctionType
    ALU = mybir.AluOpType
    PM = mybir.MatmulPerfMode
    F32 = mybir.dt.float32
    F16 = mybir.dt.float16
    F8 = mybir.dt.float8e4

    nchunk = bs // chunk
    nm = chunk // P  # batch subtiles per chunk
    NM = bs // P     # total batch subtiles

    nc = bacc.Bacc("TRN2", target_bir_lowering=False, debug=False,
                   num_devices=N_CORES)

    xt_d = nc.dram_tensor("xt", [CAT, bs], F16, kind="ExternalInput").ap()
    wbt_d = nc.dram_tensor("wbt", [CAT, UNITS], F16, kind="ExternalInput").ap()
    whf_d = nc.dram_tensor("whf", [2, UNITS, HID], F16, kind="ExternalInput").ap()
    wh8_d = nc.dram_tensor("wh8", [2, NJ, P, 2, HID], F8, kind="ExternalInput").ap()
    bbp_d = nc.dram_tensor("bbp", [P, NU], F32, kind="ExternalInput").ap()
    bhf_d = nc.dram_tensor("bhf", [2, P, HID], F32, kind="ExternalInput").ap()
    bh8_d = nc.dram_tensor("bh8", [2, P, HID], F32, kind="ExternalInput").ap()
    tsp_d = nc.dram_tensor("tsp", [P, NM], F32, kind="ExternalInput").ap()
    out_d = nc.dram_tensor("out", [bs, HID], F32, kind="ExternalOutput").ap()

    with tile.TileContext(nc) as tc:
        with (
            tc.tile_pool(name="const", bufs=1) as const,
            tc.tile_pool(name="xp", bufs=4) as xp,
            tc.tile_pool(name="hp", bufs=1) as hp,
            tc.tile_pool(name="tp", bufs=2) as tp,
            tc.tile_pool(name="op", bufs=3) as op,
            tc.tile_pool(name="psp", bufs=8, space="PSUM") as psp,
        ):
            # --- PE warmup: ramp the p-state while startup DMAs stream ----
            warm = const.tile([P, 512], F16, tag="warm")
            nc.vector.memset(warm[:], 0.0)
            for _ in range(3):
                wps = psp.tile([P, 512], F32, tag="ps")
                nc.tensor.matmul(wps[:], warm[:, 0:P], warm[:],
                                 start=True, stop=True)

            def load_x(bc):
                xts = []
                for c in range(NK1):
                    t = xp.tile([P, chunk], F16, tag=f"x{c}")
                    nc.sync.dma_start(
                        t[:], xt_d[c * P:(c + 1) * P, bc * chunk:(bc + 1) * chunk])
                    xts.append(t)
                return xts

            # first-chunk x tiles on the Sync DGE queue, layer-1 weights on
            # the Scalar DGE queue (idle until layer-1 ACT ~18us in): both
            # issue streams run in parallel so chunk 0 is fully resident
            # ~3.5us sooner than a single serialized queue
            HALF = UNITS // 2
            wb_sb = [[None, None] for _ in range(NK1)]
            xts0 = []
            for c in range(NK1):
                t = xp.tile([P, chunk], F16, tag=f"x{c}")
                # alternate the big x transfers across both queues so the
                # c-outer accumulation group never outruns the DMAs
                (nc.sync if c % 2 == 0 else nc.scalar).dma_start(
                    t[:], xt_d[c * P:(c + 1) * P, 0:chunk])
                xts0.append(t)
                t = const.tile([P, HALF], F16, tag=f"wbh{c}_0")
                (nc.scalar if c % 2 == 0 else nc.sync).dma_start(
                    t[:], wbt_d[c * P:(c + 1) * P, 0:HALF])
                wb_sb[c][0] = t

            # small constants early (bb gates every layer-1 activation)
            bb_sb = const.tile([P, NU], F32, tag="bb")
            nc.scalar.dma_start(bb_sb[:], bbp_d[:])

            for c in range(NK1):
                t = const.tile([P, HALF], F16, tag=f"wbh{c}_1")
                nc.scalar.dma_start(
                    t[:], wbt_d[c * P:(c + 1) * P, HALF:UNITS])
                wb_sb[c][1] = t

            # all remaining x chunks next: layer-1 for every chunk runs
            # before any layer-2, so the head weights are needed only ~45us in
            xts_all = [xts0] + [load_x(bc) for bc in range(1, nchunk)]

            # t-head fp8 weights (needed first in each layer-2 m-tile)
            wh8_sb = [[None] * NJ for _ in range(2)]
            for k in range(2):
                for j in range(NJ):
                    t = const.tile([P, 2, HID], F8, tag=f"wh8_{k}_{j}")
                    nc.sync.dma_start(t[:], wh8_d[k, j])
                    wh8_sb[k][j] = t

            # ff-head fp16 weights
            whf_sb = [[None] * NU for _ in range(2)]
            for k in range(2):
                for u in range(NU):
                    t = const.tile([P, HID], F16, tag=f"whf_{k}_{u}")
                    nc.sync.dma_start(t[:], whf_d[k, u * P:(u + 1) * P, :])
                    whf_sb[k][u] = t

            # biases + ts (gate the first layer-2 psum preloads / stt)
            bh8_sb = [None, None]
            bhf_sb = [None, None]
            for k in range(2):
                t = const.tile([P, HID], F32, tag=f"bh8_{k}", name=f"bh8_{k}")
                nc.sync.dma_start(t[:], bh8_d[k])
                bh8_sb[k] = t
            for k in range(2):
                t = const.tile([P, HID], F32, tag=f"bhf_{k}", name=f"bhf_{k}")
                nc.sync.dma_start(t[:], bhf_d[k])
                bhf_sb[k] = t
            ts_sb = const.tile([P, NM], F32, tag="ts")
            nc.sync.dma_start(ts_sb[:], tsp_d[:])

            # persistent h storage: fp16 per u-tile, fp8 pairs per j-tile
            h16 = [[None] * NU for _ in range(nchunk)]
            h8 = [[None] * NJ for _ in range(nchunk)]
            for bc in range(nchunk):
                for u in range(NU):
                    h16[bc][u] = hp.tile([P, chunk], F16, tag=f"h16_{bc}_{u}",
                                         name=f"h16_{bc}_{u}")
                for j in range(NJ):
                    h8[bc][j] = hp.tile([P, 2, chunk], F8, tag=f"h8_{bc}_{j}",
                                        name=f"h8_{bc}_{j}")

            def layer1(bc):
                """hT[u] = tanh(0.666*(WbT.T @ xT) + 0.666*bb), fp16 + fp8 out.

                c-outer accumulation in two u-half-groups: the first matmul
                only needs xts[0] + wb half, so PE starts as soon as the
                first ~0.26 MB of DMA lands.
                """
                xts = xts_all[bc]
                for half in range(2):
                    pss = [psp.tile([P, chunk], F32, tag="ps", name=f"psl1_{j}")
                           for j in range(NU // 2)]
                    for c in range(NK1):
                        for j in range(NU // 2):
                            nc.tensor.matmul(
                                pss[j][:],
                                wb_sb[c][half][:, j * P:(j + 1) * P],
                                xts[c][:],
                                start=(c == 0), stop=(c == NK1 - 1))
                    for j in range(NU // 2):
                        u = half * (NU // 2) + j
                        nc.scalar.activation(h16[bc][u][:], pss[j][:], AF.Tanh,
                                             bias=bb_sb[:, u:u + 1], scale=0.666)
                        nc.scalar.activation(h8[bc][u // 2][:, u % 2, :],
                                             pss[j][:], AF.Tanh,
                                             bias=bb_sb[:, u:u + 1], scale=0.666)

            def layer2(bc):
                hts = h16[bc]
                h8s = h8[bc]
                for m in range(nm):
                    mi = bc * nm + m
                    last = (bc == nchunk - 1) and (m == nm - 1)
                    mc = slice(m * P, (m + 1) * P)

                    # t-path heads (fp8 DoubleRow), biases preloaded by Pool
                    def mm_t(k):
                        ps = psp.tile([P, HID], F32, tag="ps")
                        nc.vector.tensor_copy(ps[:], bh8_sb[k][:])
                        for j in range(NJ):
                            nc.tensor.matmul(
                                ps[:],
                                h8s[j][:, :, mc],
                                wh8_sb[k][j][:],
                                start=False, stop=(j == NJ - 1),
                                perf_mode=PM.DoubleRow,
                                skip_group_check=True)
                        return ps

                    def mm_ff(k):
                        ps = psp.tile([P, HID], F32, tag="ps")
                        nc.vector.tensor_copy(ps[:], bhf_sb[k][:])
                        for u in range(NU):
                            nc.tensor.matmul(
                                ps[:],
                                hts[u][:, mc],
                                whf_sb[k][u][:],
                                start=False, stop=(u == NU - 1),
                                skip_group_check=True)
                        return ps

                    # t-path first so the sigmoid chain overlaps the ff matmuls.
                    # DVE may read only one PSUM operand per op: stage pb in
                    # SBUF via ACT (overlaps pa's matmuls).
                    pb = mm_t(1)
                    ub = tp.tile([P, HID], F32, tag="ub")
                    nc.scalar.copy(ub[:], pb[:])
                    pa = mm_t(0)
                    w = tp.tile([P, HID], F32, tag="w")
                    nc.vector.scalar_tensor_tensor(
                        w[:], pa[:], ts_sb[:, mi:mi + 1], ub[:],
                        op0=ALU.mult, op1=ALU.add)
                    tt = tp.tile([P, HID], F16, tag="tt")
                    nc.scalar.activation(tt[:], w[:], AF.Sigmoid, scale=1.0 / K8)

                    p1 = mm_ff(0)
                    f1 = tp.tile([P, HID], F16, tag="f1")
                    nc.scalar.activation(f1[:], p1[:], AF.Tanh)

                    f2 = tp.tile([P, HID], F16, tag="f2")
                    o = op.tile([P, HID], F32, tag="o")

                    def combine(cs, qi, p2t, lo):
                        """o[:, cs] = f1 + tt*(f2 - f1); p2t covers cols lo:."""
                        ls = slice(cs.start - lo, cs.stop - lo)
                        d = tp.tile([P, HID], F16, tag="d")
                        nc.scalar.activation(f2[:, cs], p2t[:, ls], AF.Tanh)
                        nc.vector.tensor_sub(d[:, cs], f2[:, cs], f1[:, cs])
                        nc.vector.tensor_mul(d[:, cs], d[:, cs], tt[:, cs])
                        nc.vector.tensor_add(o[:, cs], d[:, cs], f1[:, cs])
                        # alternate DGE queues so the tail quarters' DMA
                        # issues (~600ns each) overlap instead of serializing
                        eng = nc.scalar if (last and qi % 2) else nc.sync
                        eng.dma_start(out_d[mi * P:(mi + 1) * P, cs], o[:, cs])

                    if not last:
                        p2 = mm_ff(1)
                        combine(slice(0, HID), 0, p2, 0)
                    else:
                        # last tile: ff2 in two 256-col half-groups so the
                        # first half's chain overlaps the second half's
                        # matmuls, and quartered chains pipeline ACT/DVE/DMA
                        H2 = HID // 2
                        for hi in range(2):
                            hs = slice(hi * H2, (hi + 1) * H2)
                            ps2 = psp.tile([P, HID], F32, tag="ps")
                            nc.vector.tensor_copy(ps2[:, 0:H2], bhf_sb[1][:, hs])
                            for u in range(NU):
                                nc.tensor.matmul(
                                    ps2[:, 0:H2],
                                    hts[u][:, mc],
                                    whf_sb[1][u][:, hs],
                                    start=False, stop=(u == NU - 1),
                                    skip_group_check=True)
                            for q in range(2):
                                cs = slice(hi * H2 + q * (H2 // 2),
                                           hi * H2 + (q + 1) * (H2 // 2))
                                combine(cs, hi * 2 + q, ps2[:, 0:H2], hi * H2)

            # --- all layer-1 chunks first, then all layer-2 --------------
            for bc in range(nchunk):
                layer1(bc)
            for bc in range(nchunk):
                layer2(bc)

    nc.compile()
    return nc


def _prep_inputs(input, hx, ts, Wb, bb, W1, b1, W2, b2, Wa, ba, Wt, bt, bs=BS,
                 n_cores=N_CORES):
    import ml_dtypes
    f = np.float32
    h = np.float16
    f8 = ml_dtypes.float8_e4m3
    x = np.concatenate([np.asarray(input, f), np.asarray(hx, f)], axis=1)
    WbT = np.ascontiguousarray(np.asarray(Wb, f).T.astype(h))   # [768, 1024]
    WHF = np.stack([np.ascontiguousarray((1.7159 * np.asarray(W, f)).T.astype(h))
                    for W in (W1, W2)])                         # [2, 1024, 512]
    # fp8 DoubleRow pair layout: unit u = j*256 + s*128 + p -> [j, p, s, hid]
    WH8 = np.stack([
        np.ascontiguousarray(
            (K8 * 1.7159 * np.asarray(W, f)).T
            .reshape(NJ, 2, P, HID).transpose(0, 2, 1, 3).astype(f8))
        for W in (Wa, Wt)])                                     # [2, 4, 128, 2, 512]
    BBP = np.ascontiguousarray(
        (0.666 * np.asarray(bb, f)).reshape(NU, P).T)           # [128, 8]
    BHF = np.stack([np.ascontiguousarray(np.broadcast_to(np.asarray(b, f), (P, HID)))
                    for b in (b1, b2)])                         # [2, 128, 512]
    BH8 = np.stack([np.ascontiguousarray(np.broadcast_to(
        (K8 * np.asarray(b, f)).astype(f), (P, HID)))
        for b in (ba, bt)])                                     # [2, 128, 512]
    ts = np.asarray(ts, f).reshape(-1)
    xh = x.astype(h)

    in_maps = []
    for c in range(n_cores):
        lo, hi = c * bs, (c + 1) * bs
        in_maps.append({
            "xt": np.ascontiguousarray(xh[lo:hi].T),            # [768, bs] fp16
            "wbt": WbT,
            "whf": WHF,
            "wh8": WH8,
            "bbp": BBP,
            "bhf": BHF,
            "bh8": BH8,
            "tsp": np.ascontiguousarray(ts[lo:hi].reshape(bs // P, P).T),
        })
    return in_maps


def kernel(input, hx, ts, Wb, bb, W1, b1, W2, b2, Wa, ba, Wt, bt):
    from concourse.bass_utils import run_bass_kernel_spmd

    if "nc" not in _cache:
        _cache["nc"] = build_nc()
    nc = _cache["nc"]

    in_maps = _prep_inputs(input, hx, ts, Wb, bb, W1, b1, W2, b2, Wa, ba, Wt, bt)
    trace = bool(int(os.environ.get("KERNEL_PROFILE", "0")))
    res = run_bass_kernel_spmd(nc, in_maps, list(range(N_CORES)), trace=trace)
    _cache["last_exec_time_ns"] = res.exec_time_ns
    _cache["last_results"] = res

    out = np.concatenate([res.results[c]["out"] for c in range(N_CORES)], axis=0)
    return out.astype(np.float32)
